# revision 1
# baseline (speedup 1.0000x reference)
"""MiniMax-M2 sparse MoE block (top-2 of 8 experts, SwiGLU) on 8 Trainium2 cores.

Strategy (expert-parallel, per the sharding hint):
  - Phase 1 (device, 8 cores data-parallel over tokens): router logits in
    full fp32 (accuracy matters: a top-2 flip vs the reference would cause a
    full-scale output error on that token).
  - Host: softmax + top-2 + normalize; build per-expert token dispatch
    (the "all-to-all"), padded to a common capacity C.
  - Phase 2 (device, 1 expert per core): SwiGLU in f32r (TF32-like, full PE
    speed, ~2e-4 rel err) over the expert's gathered tokens, scaled by the
    combine weights on-chip.
  - Host: scatter-add per-expert outputs back to token order.

All device matmuls run in a transposed layout (features on partitions,
tokens on the free dim) so no on-device transposes are needed; the host
pre-transposes x and the weights instead (DMA transpose does not support
4-byte dtypes, and PE-transposing 134 MB would cost ~0.5 ms of PE time).
"""
import sys
import numpy as np

sys.path.insert(0, '/opt/trn_rl_repo')

import concourse.bass as bass  # noqa: E402
from concourse import bacc  # noqa: E402
import concourse.mybir as mybir  # noqa: E402
from concourse.tile import TileContext  # noqa: E402
from concourse.bass_utils import run_bass_kernel_spmd  # noqa: E402

F32 = mybir.dt.float32
F32R = mybir.dt.float32r

B, S, H, I, E, TOP_K = 4, 4096, 2048, 1024, 8, 2
T = B * S            # 16384 tokens
TS = T // 8          # tokens per core for phase 1
KH = H // 128        # k-chunks over H
MI = I // 128        # chunks over I
NH = H // 128        # h-chunks over H (output)

_cache = {}


def _build_phase1():
    """Per core: logitsT[E, TS] = gate_w @ xT_slice, full fp32."""
    nc = bacc.Bacc()
    xT = nc.declare_dram_parameter("xT", [H, TS], F32, isOutput=False)
    gwT = nc.declare_dram_parameter("gwT", [H, E], F32, isOutput=False)
    logT = nc.declare_dram_parameter("logT", [E, TS], F32, isOutput=True)
    NT = 512
    n_tiles = TS // NT
    with TileContext(nc) as tc:
        with tc.tile_pool(name="w", bufs=1) as wp, \
             tc.tile_pool(name="x", bufs=3) as xp, \
             tc.tile_pool(name="o", bufs=3) as op, \
             tc.tile_pool(name="ps", bufs=2, space="PSUM") as pp:
            gw_sb = wp.tile([128, KH, E], F32, tag="gw")
            for k in range(KH):
                nc.sync.dma_start(out=gw_sb[:, k, :], in_=gwT[k*128:(k+1)*128, :])
            for t in range(n_tiles):
                x_sb = xp.tile([128, KH, NT], F32, tag="x")
                for k in range(KH):
                    nc.sync.dma_start(out=x_sb[:, k, :], in_=xT[k*128:(k+1)*128, t*NT:(t+1)*NT])
                ps = pp.tile([E, NT], F32, tag="ps")
                for k in range(KH):
                    nc.tensor.matmul(ps, gw_sb[:, k, :], x_sb[:, k, :],
                                     start=(k == 0), stop=(k == KH - 1))
                o_sb = op.tile([E, NT], F32, tag="o")
                nc.vector.tensor_copy(o_sb, ps)
                nc.sync.dma_start(out=logT[:, t*NT:(t+1)*NT], in_=o_sb)
    nc.finalize()
    return nc


def _build_phase2(C, TN):
    """Per core (1 expert): yT[H, C] = wv * (w2 @ (silu(w1@xT) * (w3@xT)))."""
    assert C % TN == 0
    nc = bacc.Bacc()
    xT = nc.declare_dram_parameter("xT", [H, C], F32, isOutput=False)
    w1t = nc.declare_dram_parameter("w1t", [H, I], F32, isOutput=False)
    w3t = nc.declare_dram_parameter("w3t", [H, I], F32, isOutput=False)
    w2t = nc.declare_dram_parameter("w2t", [I, H], F32, isOutput=False)
    wv = nc.declare_dram_parameter("wv", [128, C], F32, isOutput=False)
    yT = nc.declare_dram_parameter("yT", [H, C], F32, isOutput=True)

    n_tiles = C // TN
    w2r = w2t.rearrange("(i p) h -> p i h", p=128)  # [128, MI, H]

    with TileContext(nc) as tc:
        with tc.tile_pool(name="wres", bufs=1) as wres, \
             tc.tile_pool(name="xs", bufs=1) as xs, \
             tc.tile_pool(name="gs", bufs=1) as gs, \
             tc.tile_pool(name="w2s", bufs=3) as w2s, \
             tc.tile_pool(name="sm", bufs=3) as sm, \
             tc.tile_pool(name="ps13", bufs=2, space="PSUM") as ps13, \
             tc.tile_pool(name="pso", bufs=2, space="PSUM") as pso:
            w1_sb = wres.tile([128, KH, I], F32R, tag="w1")
            w3_sb = wres.tile([128, KH, I], F32R, tag="w3")
            for k in range(KH):
                nc.sync.dma_start(out=w1_sb[:, k, :], in_=w1t[k*128:(k+1)*128, :].bitcast(F32R))
                nc.sync.dma_start(out=w3_sb[:, k, :], in_=w3t[k*128:(k+1)*128, :].bitcast(F32R))

            for t in range(n_tiles):
                tsl = slice(t*TN, (t+1)*TN)
                x_sb = xs.tile([128, KH, TN], F32R, tag="x")
                for k in range(KH):
                    nc.sync.dma_start(out=x_sb[:, k, :], in_=xT[k*128:(k+1)*128, tsl].bitcast(F32R))
                wv_sb = sm.tile([128, TN], F32, tag="wv")
                nc.sync.dma_start(out=wv_sb, in_=wv[:, tsl])

                g_sb = gs.tile([128, MI, TN], F32R, tag="g")
                for m in range(MI):
                    p1 = ps13.tile([128, TN], F32, tag="p1")
                    for k in range(KH):
                        nc.tensor.matmul(p1, w1_sb[:, k, m*128:(m+1)*128], x_sb[:, k, :],
                                         start=(k == 0), stop=(k == KH - 1))
                    p3 = ps13.tile([128, TN], F32, tag="p3")
                    for k in range(KH):
                        nc.tensor.matmul(p3, w3_sb[:, k, m*128:(m+1)*128], x_sb[:, k, :],
                                         start=(k == 0), stop=(k == KH - 1))
                    s_sb = sm.tile([128, TN], F32, tag="s")
                    nc.scalar.activation(s_sb, p1, mybir.ActivationFunctionType.Silu)
                    nc.vector.tensor_mul(g_sb[:, m, :], s_sb, p3)

                for h in range(NH):
                    w2_sb = w2s.tile([128, MI, 128], F32R, tag="w2")
                    nc.sync.dma_start(out=w2_sb, in_=w2r[:, :, h*128:(h+1)*128].bitcast(F32R))
                    po = pso.tile([128, TN], F32, tag="po")
                    for i in range(MI):
                        nc.tensor.matmul(po, w2_sb[:, i, :], g_sb[:, i, :],
                                         start=(i == 0), stop=(i == MI - 1))
                    o_sb = sm.tile([128, TN], F32, tag="o")
                    nc.vector.tensor_mul(o_sb, po, wv_sb)
                    nc.sync.dma_start(out=yT[h*128:(h+1)*128, tsl], in_=o_sb)
    nc.finalize()
    return nc


def kernel(hidden_states, gate_w, w1, w2, w3):
    TN = 384
    hidden_states = np.asarray(hidden_states, dtype=np.float32)
    gate_w = np.asarray(gate_w, dtype=np.float32)
    w1 = np.asarray(w1, dtype=np.float32)
    w2 = np.asarray(w2, dtype=np.float32)
    w3 = np.asarray(w3, dtype=np.float32)

    x = np.ascontiguousarray(hidden_states.reshape(T, H))
    xT = np.ascontiguousarray(x.T)                 # [H, T]
    gwT = np.ascontiguousarray(gate_w.T)           # [H, E]
    core_ids = list(range(8))

    # ---- phase 1: router logits on device ----
    if "p1" not in _cache:
        _cache["p1"] = _build_phase1()
    nc1 = _cache["p1"]
    in_maps1 = [{"xT": np.ascontiguousarray(xT[:, c*TS:(c+1)*TS]), "gwT": gwT}
                for c in core_ids]
    r1 = run_bass_kernel_spmd(nc1, in_maps1, core_ids)
    logits = np.concatenate([r1.results[c]["logT"].T for c in core_ids], axis=0)

    # ---- host routing: softmax + top-2 + normalize, build dispatch ----
    lg = logits.astype(np.float32)
    m = lg.max(axis=-1, keepdims=True)
    p = np.exp(lg - m)
    p /= p.sum(axis=-1, keepdims=True)
    sel = np.argsort(-p, axis=-1, kind="stable")[:, :TOP_K]
    topv = np.take_along_axis(p, sel, axis=-1)
    rw = (topv / np.clip(topv.sum(axis=-1, keepdims=True), 1e-12, None)).astype(np.float32)

    idx_e, wv_e = [], []
    for e in range(E):
        mask = (sel == e)
        tok = np.nonzero(mask.any(axis=-1))[0]
        idx_e.append(tok)
        wv_e.append((rw * mask).sum(axis=-1)[tok].astype(np.float32))
    counts = [len(ix) for ix in idx_e]
    C = ((max(counts) + TN - 1) // TN) * TN

    # ---- phase 2: expert-parallel SwiGLU on device ----
    key = ("p2", C, TN)
    if key not in _cache:
        _cache[key] = _build_phase2(C, TN)
    nc2 = _cache[key]
    in_maps2 = []
    for e in range(E):
        xTe = np.zeros((H, C), np.float32)
        xTe[:, :counts[e]] = np.take(xT, idx_e[e], axis=1)
        wvb = np.zeros((128, C), np.float32)
        wvb[:, :counts[e]] = wv_e[e][None, :]
        in_maps2.append({
            "xT": xTe,
            "w1t": np.ascontiguousarray(w1[e].T),
            "w3t": np.ascontiguousarray(w3[e].T),
            "w2t": np.ascontiguousarray(w2[e].T),
            "wv": wvb,
        })
    r2 = run_bass_kernel_spmd(nc2, in_maps2, core_ids)

    # ---- host combine (scatter-add; indices unique within one expert) ----
    out = np.zeros((T, H), np.float32)
    for e in range(E):
        out[idx_e[e]] += r2.results[e]["yT"][:, :counts[e]].T
    return out.reshape(B, S, H), logits


# revision 2
# speedup vs baseline: 1.1423x; 1.1423x over previous
"""MiniMax-M2 sparse MoE block (top-2 of 8 experts, SwiGLU) on 8 Trainium2 cores.

Strategy (expert-parallel, per the sharding hint):
  - Phase 1 (device, 8 cores data-parallel over tokens): router logits in
    full fp32 (accuracy matters: a top-2 flip vs the reference would cause a
    full-scale output error on that token).
  - Host: softmax + top-2 + normalize; build the per-expert token dispatch
    (the "all-to-all"), padded to a common capacity C (multiple of 128).
  - Phase 2 (device, 1 expert per core): SwiGLU in f32r (TF32-like matmul
    dtype: full PE speed at moving-dim >= 256, ~2e-4 rel err) over the
    expert's gathered tokens, combine weights applied on-chip.
  - Host: scatter-add per-expert outputs back to token order.

Layout: all device matmuls run transposed (features on partitions, tokens on
the free dim) so no on-device transposes are needed; the host pre-transposes
x and the weights (DMA transpose does not support 4-byte dtypes).

Phase-2 structure per core (verified at ~829 us modeled, PE-roofline 699 us):
  - w1^T, w3^T resident in SBUF (128 KB/partition), m-major chunk layout so
    the first m-group can start after ~4 MB of weight DMA instead of 16 MB.
  - w2^T streamed per token tile (one 0.5 MB DMA per output h-chunk) on the
    SP HWDGE queue; x / wv / y traffic runs on the Activation HWDGE queue so
    the x prefetch is not head-of-line blocked by weight or output DMAs.
  - g (SwiGLU intermediate) double-buffered so stage 2 of tile t overlaps
    stage 1 of tile t+1, spreading the w2 stream over the whole tile period.
  - Token tiles of 384 (plus 256-token tiles to absorb C % 384) keep every
    f32r matmul moving-dim >= 256 (below 256 f32r drops to 1/4 rate).
"""
import sys
import numpy as np

sys.path.insert(0, '/opt/trn_rl_repo')

import concourse.bass as bass  # noqa: E402
from concourse import bacc  # noqa: E402
import concourse.mybir as mybir  # noqa: E402
from concourse.tile import TileContext  # noqa: E402
from concourse.bass_utils import run_bass_kernel_spmd  # noqa: E402

F32 = mybir.dt.float32
F32R = mybir.dt.float32r

B, S, H, I, E, TOP_K = 4, 4096, 2048, 1024, 8, 2
T = B * S            # 16384 tokens
TS = T // 8          # tokens per core for phase 1
KH = H // 128        # k-chunks over H
MI = I // 128        # chunks over I
NH = H // 128        # h-chunks over H (output)

_cache = {}


def _build_phase1():
    """Per core: logitsT[E, TS] = gate_w @ xT_slice, full fp32."""
    nc = bacc.Bacc()
    xT = nc.declare_dram_parameter("xT", [H, TS], F32, isOutput=False)
    gwT = nc.declare_dram_parameter("gwT", [H, E], F32, isOutput=False)
    logT = nc.declare_dram_parameter("logT", [E, TS], F32, isOutput=True)
    NT = 512
    n_tiles = TS // NT
    with TileContext(nc) as tc:
        with tc.tile_pool(name="w", bufs=1) as wp, \
             tc.tile_pool(name="x", bufs=3) as xp, \
             tc.tile_pool(name="o", bufs=3) as op, \
             tc.tile_pool(name="ps", bufs=2, space="PSUM") as pp:
            gwr = gwT.rearrange("(k p) e -> p k e", p=128)
            gw_sb = wp.tile([128, KH, E], F32, tag="gw")
            nc.sync.dma_start(out=gw_sb, in_=gwr)
            for t in range(n_tiles):
                x_sb = xp.tile([128, KH, NT], F32, tag="x")
                for k in range(KH):
                    nc.scalar.dma_start(out=x_sb[:, k, :], in_=xT[k*128:(k+1)*128, t*NT:(t+1)*NT])
                ps = pp.tile([E, NT], F32, tag="ps")
                for k in range(KH):
                    nc.tensor.matmul(ps, gw_sb[:, k, :], x_sb[:, k, :],
                                     start=(k == 0), stop=(k == KH - 1))
                o_sb = op.tile([E, NT], F32, tag="o")
                nc.vector.tensor_copy(o_sb, ps)
                nc.sync.dma_start(out=logT[:, t*NT:(t+1)*NT], in_=o_sb)
    nc.finalize()
    return nc


def _build_phase2(C, TN=384, g_bufs=2, w2_bufs=2):
    """Per core (1 expert): yT[H, C] = wv * (w2 @ (silu(w1@xT) * (w3@xT)))."""
    assert C % 128 == 0 and C >= 512
    n384 = C // TN
    rem = C - n384 * TN          # 0, 128, or 256 for TN=384
    if rem == 0:
        sizes = [TN] * n384
    elif rem == 128:
        sizes = [256, 256] + [TN] * (n384 - 1)
    else:
        sizes = [rem] + [TN] * n384
    assert sum(sizes) == C and all(s >= 256 for s in sizes)
    starts = [sum(sizes[:i]) for i in range(len(sizes))]

    nc = bacc.Bacc()
    xT = nc.declare_dram_parameter("xT", [H, C], F32, isOutput=False)
    w1t = nc.declare_dram_parameter("w1t", [H, I], F32, isOutput=False)
    w3t = nc.declare_dram_parameter("w3t", [H, I], F32, isOutput=False)
    w2t = nc.declare_dram_parameter("w2t", [I, H], F32, isOutput=False)
    wv = nc.declare_dram_parameter("wv", [128, C], F32, isOutput=False)
    yT = nc.declare_dram_parameter("yT", [H, C], F32, isOutput=True)

    n_tiles = len(sizes)
    w1r = w1t.rearrange("(k p) i -> p k i", p=128)
    w3r = w3t.rearrange("(k p) i -> p k i", p=128)
    w2r = w2t.rearrange("(i p) h -> p i h", p=128)

    with TileContext(nc) as tc:
        with tc.tile_pool(name="wres", bufs=1) as wres, \
             tc.tile_pool(name="xs", bufs=1) as xs, \
             tc.tile_pool(name="gs", bufs=g_bufs) as gs, \
             tc.tile_pool(name="w2s", bufs=w2_bufs) as w2s, \
             tc.tile_pool(name="sm", bufs=2) as sm, \
             tc.tile_pool(name="op", bufs=3) as op_pool, \
             tc.tile_pool(name="ps13", bufs=2, space="PSUM") as ps13, \
             tc.tile_pool(name="pso", bufs=2, space="PSUM") as pso:
            x_tiles = {}
            x_first = xs.tile([128, KH, TN], F32R, tag="x")
            for k in range(KH):
                nc.scalar.dma_start(out=x_first[:, k, :sizes[0]],
                                    in_=xT[k*128:(k+1)*128, 0:sizes[0]].bitcast(F32R))
            w1_sb = wres.tile([128, MI, KH, 128], F32R, tag="w1")
            w3_sb = wres.tile([128, MI, KH, 128], F32R, tag="w3")
            for m in range(MI):
                nc.sync.dma_start(out=w1_sb[:, m, :, :], in_=w1r[:, :, m*128:(m+1)*128].bitcast(F32R))
                nc.sync.dma_start(out=w3_sb[:, m, :, :], in_=w3r[:, :, m*128:(m+1)*128].bitcast(F32R))

            for t in range(n_tiles):
                tn = sizes[t]
                t0 = starts[t]
                tsl = slice(t0, t0 + tn)
                x_sb = x_first if t == 0 else x_tiles.pop(t)
                wv_sb = sm.tile([128, TN], F32, tag="wv")
                nc.scalar.dma_start(out=wv_sb[:, :tn], in_=wv[:, tsl])

                g_sb = gs.tile([128, MI, TN], F32R, tag="g")
                for m in range(MI):
                    p1 = ps13.tile([128, TN], F32, tag="p1")
                    for k in range(KH):
                        nc.tensor.matmul(p1[:, :tn], w1_sb[:, m, k, :], x_sb[:, k, :tn],
                                         start=(k == 0), stop=(k == KH - 1))
                    p3 = ps13.tile([128, TN], F32, tag="p3")
                    for k in range(KH):
                        nc.tensor.matmul(p3[:, :tn], w3_sb[:, m, k, :], x_sb[:, k, :tn],
                                         start=(k == 0), stop=(k == KH - 1))
                    s_sb = sm.tile([128, TN], F32, tag="s")
                    nc.scalar.activation(s_sb[:, :tn], p1[:, :tn], mybir.ActivationFunctionType.Silu)
                    nc.vector.tensor_mul(g_sb[:, m, :tn], s_sb[:, :tn], p3[:, :tn])

                if t + 1 < n_tiles:
                    nx = xs.tile([128, KH, TN], F32R, tag="x")
                    nsl = slice(starts[t+1], starts[t+1] + sizes[t+1])
                    for k in range(KH):
                        nc.scalar.dma_start(out=nx[:, k, :sizes[t+1]],
                                            in_=xT[k*128:(k+1)*128, nsl].bitcast(F32R))
                    x_tiles[t + 1] = nx

                for h in range(NH):
                    w2_sb = w2s.tile([128, MI, 128], F32R, tag="w2")
                    nc.sync.dma_start(out=w2_sb, in_=w2r[:, :, h*128:(h+1)*128].bitcast(F32R))
                    po = pso.tile([128, TN], F32, tag="po")
                    for i in range(MI):
                        nc.tensor.matmul(po[:, :tn], w2_sb[:, i, :], g_sb[:, i, :tn],
                                         start=(i == 0), stop=(i == MI - 1))
                    o_sb = op_pool.tile([128, TN], F32, tag="o")
                    nc.vector.tensor_mul(o_sb[:, :tn], po[:, :tn], wv_sb[:, :tn])
                    nc.scalar.dma_start(out=yT[h*128:(h+1)*128, tsl], in_=o_sb[:, :tn])
    nc.finalize()
    return nc


def kernel(hidden_states, gate_w, w1, w2, w3):
    TN = 384
    hidden_states = np.asarray(hidden_states, dtype=np.float32)
    gate_w = np.asarray(gate_w, dtype=np.float32)
    w1 = np.asarray(w1, dtype=np.float32)
    w2 = np.asarray(w2, dtype=np.float32)
    w3 = np.asarray(w3, dtype=np.float32)

    x = np.ascontiguousarray(hidden_states.reshape(T, H))
    xT = np.ascontiguousarray(x.T)                 # [H, T]
    gwT = np.ascontiguousarray(gate_w.T)           # [H, E]
    core_ids = list(range(8))

    # ---- phase 1: router logits on device ----
    if "p1" not in _cache:
        _cache["p1"] = _build_phase1()
    in_maps1 = [{"xT": np.ascontiguousarray(xT[:, c*TS:(c+1)*TS]), "gwT": gwT}
                for c in core_ids]
    r1 = run_bass_kernel_spmd(_cache["p1"], in_maps1, core_ids)
    logits = np.concatenate([r1.results[c]["logT"].T for c in core_ids], axis=0)

    # ---- host routing: softmax + top-2 + normalize, build dispatch ----
    lg = logits.astype(np.float32)
    p = np.exp(lg - lg.max(axis=-1, keepdims=True))
    p /= p.sum(axis=-1, keepdims=True)
    sel = np.argsort(-p, axis=-1, kind="stable")[:, :TOP_K]
    topv = np.take_along_axis(p, sel, axis=-1)
    rw = (topv / np.clip(topv.sum(axis=-1, keepdims=True), 1e-12, None)).astype(np.float32)

    idx_e, wv_e = [], []
    for e in range(E):
        mask = (sel == e)
        tok = np.nonzero(mask.any(axis=-1))[0]
        idx_e.append(tok)
        wv_e.append((rw * mask).sum(axis=-1)[tok].astype(np.float32))
    counts = [len(ix) for ix in idx_e]
    C = max(512, ((max(counts) + 127) // 128) * 128)

    # ---- phase 2: expert-parallel SwiGLU on device ----
    key = ("p2", C, TN)
    if key not in _cache:
        _cache[key] = _build_phase2(C, TN)
    in_maps2 = []
    for e in range(E):
        xTe = np.zeros((H, C), np.float32)
        xTe[:, :counts[e]] = np.take(xT, idx_e[e], axis=1)
        wvb = np.zeros((128, C), np.float32)
        wvb[:, :counts[e]] = wv_e[e][None, :]
        in_maps2.append({
            "xT": xTe,
            "w1t": np.ascontiguousarray(w1[e].T),
            "w3t": np.ascontiguousarray(w3[e].T),
            "w2t": np.ascontiguousarray(w2[e].T),
            "wv": wvb,
        })
    r2 = run_bass_kernel_spmd(_cache[key], in_maps2, core_ids)

    # ---- host combine (scatter-add; indices unique within one expert) ----
    out = np.zeros((T, H), np.float32)
    for e in range(E):
        out[idx_e[e]] += r2.results[e]["yT"][:, :counts[e]].T
    return out.reshape(B, S, H), logits


# revision 4
# speedup vs baseline: 1.1634x; 1.0185x over previous
"""MiniMax-M2 sparse MoE block (top-2 of 8 experts, SwiGLU) on 8 Trainium2 cores.

Strategy (expert-parallel, per the sharding hint):
  - Phase 1 (device, 8 cores data-parallel over tokens): router logits in
    full fp32 (accuracy matters: a top-2 flip vs the reference would cause a
    full-scale output error on that token).
  - Host: softmax + top-2 + normalize; build the per-expert token dispatch
    (the "all-to-all"), padded to a common capacity C (multiple of 128).
  - Phase 2 (device, 1 expert per core): SwiGLU in f32r (TF32-like matmul
    dtype: full PE speed at moving-dim >= 256, ~2e-4 rel err) over the
    expert's gathered tokens, combine weights applied on-chip.
  - Host: scatter-add per-expert outputs back to token order.

Layout: all device matmuls run transposed (features on partitions, tokens on
the free dim) so no on-device transposes are needed; the host pre-transposes
x and the weights (DMA transpose does not support 4-byte dtypes).

Phase-2 structure per core (verified at ~829 us modeled, PE-roofline 699 us):
  - w1^T, w3^T resident in SBUF (128 KB/partition), m-major chunk layout so
    the first m-group can start after ~4 MB of weight DMA instead of 16 MB.
  - w2^T streamed per token tile (one 0.5 MB DMA per output h-chunk) on the
    SP HWDGE queue; x / wv / y traffic runs on the Activation HWDGE queue so
    the x prefetch is not head-of-line blocked by weight or output DMAs.
  - g (SwiGLU intermediate) double-buffered so stage 2 of tile t overlaps
    stage 1 of tile t+1, spreading the w2 stream over the whole tile period.
  - Token tiles of 448 (plus 256-512 tiles absorbing the remainder) keep every
    f32r matmul moving-dim >= 256 (below 256 f32r drops to 1/4 rate).
"""
import sys
import numpy as np

sys.path.insert(0, '/opt/trn_rl_repo')

import concourse.bass as bass  # noqa: E402
from concourse import bacc  # noqa: E402
import concourse.mybir as mybir  # noqa: E402
from concourse.tile import TileContext  # noqa: E402
from concourse.bass_utils import run_bass_kernel_spmd  # noqa: E402

F32 = mybir.dt.float32
F32R = mybir.dt.float32r

B, S, H, I, E, TOP_K = 4, 4096, 2048, 1024, 8, 2
T = B * S            # 16384 tokens
TS = T // 8          # tokens per core for phase 1
KH = H // 128        # k-chunks over H
MI = I // 128        # chunks over I
NH = H // 128        # h-chunks over H (output)

_cache = {}


def _build_phase1():
    """Per core: logitsT[E, TS] = gate_w @ xT_slice, full fp32."""
    nc = bacc.Bacc()
    xT = nc.declare_dram_parameter("xT", [H, TS], F32, isOutput=False)
    gwT = nc.declare_dram_parameter("gwT", [H, E], F32, isOutput=False)
    logT = nc.declare_dram_parameter("logT", [E, TS], F32, isOutput=True)
    NT = 512
    n_tiles = TS // NT
    with TileContext(nc) as tc:
        with tc.tile_pool(name="w", bufs=1) as wp, \
             tc.tile_pool(name="x", bufs=3) as xp, \
             tc.tile_pool(name="o", bufs=3) as op, \
             tc.tile_pool(name="ps", bufs=2, space="PSUM") as pp:
            gwr = gwT.rearrange("(k p) e -> p k e", p=128)
            gw_sb = wp.tile([128, KH, E], F32, tag="gw")
            nc.sync.dma_start(out=gw_sb, in_=gwr)
            for t in range(n_tiles):
                x_sb = xp.tile([128, KH, NT], F32, tag="x")
                for k in range(KH):
                    nc.scalar.dma_start(out=x_sb[:, k, :], in_=xT[k*128:(k+1)*128, t*NT:(t+1)*NT])
                ps = pp.tile([E, NT], F32, tag="ps")
                for k in range(KH):
                    nc.tensor.matmul(ps, gw_sb[:, k, :], x_sb[:, k, :],
                                     start=(k == 0), stop=(k == KH - 1))
                o_sb = op.tile([E, NT], F32, tag="o")
                nc.vector.tensor_copy(o_sb, ps)
                nc.sync.dma_start(out=logT[:, t*NT:(t+1)*NT], in_=o_sb)
    nc.finalize()
    return nc


def _build_phase2(C, TN=384, g_bufs=2, w2_bufs=2):
    """Per core (1 expert): yT[H, C] = wv * (w2 @ (silu(w1@xT) * (w3@xT)))."""
    assert C % 128 == 0 and C >= 512
    n_full = C // TN
    rem = C - n_full * TN
    if rem == 0:
        sizes = [TN] * n_full
    elif rem >= 256:
        sizes = [rem] + [TN] * n_full
    else:
        # split one full tile + remainder into two tiles, both in [256, TN]
        sizes = [256, TN + rem - 256] + [TN] * (n_full - 1)
    assert sum(sizes) == C and all(256 <= s <= 512 for s in sizes)
    starts = [sum(sizes[:i]) for i in range(len(sizes))]

    nc = bacc.Bacc()
    xT = nc.declare_dram_parameter("xT", [H, C], F32, isOutput=False)
    w1t = nc.declare_dram_parameter("w1t", [H, I], F32, isOutput=False)
    w3t = nc.declare_dram_parameter("w3t", [H, I], F32, isOutput=False)
    w2t = nc.declare_dram_parameter("w2t", [I, H], F32, isOutput=False)
    wv = nc.declare_dram_parameter("wv", [128, C], F32, isOutput=False)
    yT = nc.declare_dram_parameter("yT", [H, C], F32, isOutput=True)

    n_tiles = len(sizes)
    w1r = w1t.rearrange("(k p) i -> p k i", p=128)
    w3r = w3t.rearrange("(k p) i -> p k i", p=128)
    w2r = w2t.rearrange("(i p) h -> p i h", p=128)

    with TileContext(nc) as tc:
        with tc.tile_pool(name="wres", bufs=1) as wres, \
             tc.tile_pool(name="xs", bufs=1) as xs, \
             tc.tile_pool(name="gs", bufs=g_bufs) as gs, \
             tc.tile_pool(name="w2s", bufs=w2_bufs) as w2s, \
             tc.tile_pool(name="sm", bufs=2) as sm, \
             tc.tile_pool(name="op", bufs=3) as op_pool, \
             tc.tile_pool(name="ps13", bufs=2, space="PSUM") as ps13, \
             tc.tile_pool(name="pso", bufs=2, space="PSUM") as pso:
            x_tiles = {}
            x_first = xs.tile([128, KH, TN], F32R, tag="x")
            for k in range(KH):
                nc.scalar.dma_start(out=x_first[:, k, :sizes[0]],
                                    in_=xT[k*128:(k+1)*128, 0:sizes[0]].bitcast(F32R))
            w1_sb = wres.tile([128, MI, KH, 128], F32R, tag="w1")
            w3_sb = wres.tile([128, MI, KH, 128], F32R, tag="w3")
            for m in range(MI):
                nc.sync.dma_start(out=w1_sb[:, m, :, :], in_=w1r[:, :, m*128:(m+1)*128].bitcast(F32R))
                nc.sync.dma_start(out=w3_sb[:, m, :, :], in_=w3r[:, :, m*128:(m+1)*128].bitcast(F32R))

            for t in range(n_tiles):
                tn = sizes[t]
                t0 = starts[t]
                tsl = slice(t0, t0 + tn)
                x_sb = x_first if t == 0 else x_tiles.pop(t)
                wv_sb = sm.tile([128, TN], F32, tag="wv")
                nc.scalar.dma_start(out=wv_sb[:, :tn], in_=wv[:, tsl])

                g_sb = gs.tile([128, MI, TN], F32R, tag="g")
                for m in range(MI):
                    p1 = ps13.tile([128, TN], F32, tag="p1")
                    for k in range(KH):
                        nc.tensor.matmul(p1[:, :tn], w1_sb[:, m, k, :], x_sb[:, k, :tn],
                                         start=(k == 0), stop=(k == KH - 1))
                    p3 = ps13.tile([128, TN], F32, tag="p3")
                    for k in range(KH):
                        nc.tensor.matmul(p3[:, :tn], w3_sb[:, m, k, :], x_sb[:, k, :tn],
                                         start=(k == 0), stop=(k == KH - 1))
                    s_sb = sm.tile([128, TN], F32, tag="s")
                    nc.scalar.activation(s_sb[:, :tn], p1[:, :tn], mybir.ActivationFunctionType.Silu)
                    nc.vector.tensor_mul(g_sb[:, m, :tn], s_sb[:, :tn], p3[:, :tn])

                if t + 1 < n_tiles:
                    nx = xs.tile([128, KH, TN], F32R, tag="x")
                    nsl = slice(starts[t+1], starts[t+1] + sizes[t+1])
                    for k in range(KH):
                        nc.scalar.dma_start(out=nx[:, k, :sizes[t+1]],
                                            in_=xT[k*128:(k+1)*128, nsl].bitcast(F32R))
                    x_tiles[t + 1] = nx

                for h in range(NH):
                    w2_sb = w2s.tile([128, MI, 128], F32R, tag="w2")
                    nc.sync.dma_start(out=w2_sb, in_=w2r[:, :, h*128:(h+1)*128].bitcast(F32R))
                    po = pso.tile([128, TN], F32, tag="po")
                    for i in range(MI):
                        nc.tensor.matmul(po[:, :tn], w2_sb[:, i, :], g_sb[:, i, :tn],
                                         start=(i == 0), stop=(i == MI - 1))
                    o_sb = op_pool.tile([128, TN], F32, tag="o")
                    nc.vector.tensor_mul(o_sb[:, :tn], po[:, :tn], wv_sb[:, :tn])
                    nc.scalar.dma_start(out=yT[h*128:(h+1)*128, tsl], in_=o_sb[:, :tn])
    nc.finalize()
    return nc


def kernel(hidden_states, gate_w, w1, w2, w3):
    TN = 448
    hidden_states = np.asarray(hidden_states, dtype=np.float32)
    gate_w = np.asarray(gate_w, dtype=np.float32)
    w1 = np.asarray(w1, dtype=np.float32)
    w2 = np.asarray(w2, dtype=np.float32)
    w3 = np.asarray(w3, dtype=np.float32)

    x = np.ascontiguousarray(hidden_states.reshape(T, H))
    xT = np.ascontiguousarray(x.T)                 # [H, T]
    gwT = np.ascontiguousarray(gate_w.T)           # [H, E]
    core_ids = list(range(8))

    # ---- phase 1: router logits on device ----
    if "p1" not in _cache:
        _cache["p1"] = _build_phase1()
    in_maps1 = [{"xT": np.ascontiguousarray(xT[:, c*TS:(c+1)*TS]), "gwT": gwT}
                for c in core_ids]
    r1 = run_bass_kernel_spmd(_cache["p1"], in_maps1, core_ids)
    logits = np.concatenate([r1.results[c]["logT"].T for c in core_ids], axis=0)

    # ---- host routing: softmax + top-2 + normalize, build dispatch ----
    lg = logits.astype(np.float32)
    p = np.exp(lg - lg.max(axis=-1, keepdims=True))
    p /= p.sum(axis=-1, keepdims=True)
    sel = np.argsort(-p, axis=-1, kind="stable")[:, :TOP_K]
    topv = np.take_along_axis(p, sel, axis=-1)
    rw = (topv / np.clip(topv.sum(axis=-1, keepdims=True), 1e-12, None)).astype(np.float32)

    idx_e, wv_e = [], []
    for e in range(E):
        mask = (sel == e)
        tok = np.nonzero(mask.any(axis=-1))[0]
        idx_e.append(tok)
        wv_e.append((rw * mask).sum(axis=-1)[tok].astype(np.float32))
    counts = [len(ix) for ix in idx_e]
    C = max(512, ((max(counts) + 127) // 128) * 128)

    # ---- phase 2: expert-parallel SwiGLU on device ----
    key = ("p2", C, TN)
    if key not in _cache:
        _cache[key] = _build_phase2(C, TN)
    in_maps2 = []
    for e in range(E):
        xTe = np.zeros((H, C), np.float32)
        xTe[:, :counts[e]] = np.take(xT, idx_e[e], axis=1)
        wvb = np.zeros((128, C), np.float32)
        wvb[:, :counts[e]] = wv_e[e][None, :]
        in_maps2.append({
            "xT": xTe,
            "w1t": np.ascontiguousarray(w1[e].T),
            "w3t": np.ascontiguousarray(w3[e].T),
            "w2t": np.ascontiguousarray(w2[e].T),
            "wv": wvb,
        })
    r2 = run_bass_kernel_spmd(_cache[key], in_maps2, core_ids)

    # ---- host combine (scatter-add; indices unique within one expert) ----
    out = np.zeros((T, H), np.float32)
    for e in range(E):
        out[idx_e[e]] += r2.results[e]["yT"][:, :counts[e]].T
    return out.reshape(B, S, H), logits
